# revision 16
# baseline (speedup 1.0000x reference)
"""GAT layer (dense-softmax graph attention) on Trainium2, 8 NeuronCores.

Math (matches the reference exactly):
    Wh    = x @ W
    s_src = Wh @ a[:F_OUT] = x @ (W @ a[:F_OUT])
    s_dst = Wh @ a[F_OUT:] = x @ (W @ a[F_OUT:])
    e_ij  = leaky_relu(s_src[i] + s_dst[j], 0.2)
    att   = softmax_row(where(adj != 0, e, 0))
    out   = (att @ Wh).reshape(N, H, F_OUT/H).mean(axis=1)
          = att @ (x @ W_headmean)            # mean commutes with att @ .

Key identities used on device:
    p_ij = exp(adj_ij * lrelu(s_src_i + s_dst_j))   (non-edge -> exp(0) = 1,
           exactly the dense-softmax behaviour of the reference)
    row numerator+denominator in one matmul via a ones column:
           [h'_i | d_i] = sum_j p_ij * [Whm_j | 1]
    out_i = h'_i / d_i

v2 vs v1: bf16 operands end-to-end (DVE 2x perf mode, PE FWL weight loads,
half the x DMA), int8 adj (quarter the adj DMA), Y chunks produced directly
in [j, col] layout via stationary-xT matmuls (no PE transposes, no wide
f32 bounce copies), quad-batched Exp, ACT/DVE path split rebalanced for
the bf16 rates.

Sharding: 1D partition of output rows i across 8 cores. Each core reads its
transposed row-slice of adj (layout [j, i]) plus all of x, and writes its
own 1024 output rows. No cross-core communication.
"""

import numpy as np
import ml_dtypes

import concourse.bacc as bacc
import concourse.tile as tile
from concourse import mybir
from concourse.bass_utils import run_bass_kernel_spmd
from concourse.masks import make_identity

P = 128
F_IN = 512
F_OUT = 256
HEADS = 4
FM = F_OUT // HEADS        # 64 folded (head-averaged) features
FC = FM + 1                # 65 columns of B: [wd | Wm]
YTC = FM + 2               # 66 columns of a Y chunk: [s_dst | Whm | ones]
KC = F_IN // P             # 4 contraction chunks
N_CORES = 8
N_FULL = 8192
LRELU_SLOPE = 0.2
F16 = np.float16


B_BATCHES = frozenset({1, 4, 6, 9, 11, 14})


def build_nc(n=N_FULL, r=None, b_batches=B_BATCHES):
    """Build the SPMD Bass program (same program on every core).

    n: total number of graph nodes; r: output rows per core.
    b_batches: adj batches (of 4 j-tiles) that run the all-DVE max-trick
    path  p = max(e^s_src*e^s_dst, e^.2s_src*e^.2s_dst)  instead of the
    ACT Prelu+Exp path; balances ACT vs DVE load.
    """
    if r is None:
        r = n // N_CORES
    assert n % P == 0 and r % P == 0
    jt_n = n // P              # number of 128-row j-chunks
    ab = 4                     # j-tiles per adj batch / Exp quad
    n_ab = jt_n // ab
    nq = 4                     # xT quarters
    jt_q = jt_n // nq
    mov = min(r, 512)          # PSUM bank limit: 512 fp32 per matmul
    mh = r // mov
    ich = r // P               # output row chunks
    f32 = mybir.dt.float32
    f16 = mybir.dt.float16
    bf16 = mybir.dt.bfloat16

    b_first = min(b_batches) if b_batches else None
    b_last = max(b_batches) if b_batches else None

    AF = mybir.ActivationFunctionType
    OP = mybir.AluOpType

    nc = bacc.Bacc(None, target_bir_lowering=False)
    # xT chunks as matmul stationaries: [p=k%128, b, kc, a, jj]
    xT_d = nc.dram_tensor("xT", [P, n_ab, KC, ab, P], f16, kind="ExternalInput")
    xsT_d = nc.dram_tensor("xsT", [P, KC, r], f16, kind="ExternalInput")
    adj_d = nc.dram_tensor("adjc", [P, n_ab, ab, r], f16, kind="ExternalInput")
    B_d = nc.dram_tensor("B", [F_IN, FC], f16, kind="ExternalInput")
    ws_d = nc.dram_tensor("wsv", [F_IN, 1], f16, kind="ExternalInput")
    h_d = nc.dram_tensor("h", [r, FM], f32, kind="ExternalOutput")

    with tile.TileContext(nc) as tc:
        with (
            tc.tile_pool(name="consts", bufs=1) as consts,
            tc.tile_pool(name="xpool", bufs=1) as xpool,
            tc.tile_pool(name="adjpool", bufs=3) as adjpool,
            tc.tile_pool(name="upool", bufs=2) as upool,
            tc.tile_pool(name="tpool", bufs=2) as tpool,
            tc.tile_pool(name="bpool", bufs=1) as bpool,
            tc.tile_pool(name="ppool", bufs=3) as ppool,
            tc.tile_pool(name="mpool", bufs=2) as mpool,
            tc.tile_pool(name="yps", bufs=2, space="PSUM") as yps,
            tc.tile_pool(name="sps", bufs=1, space="PSUM") as sps,
            tc.tile_pool(name="accps", bufs=1, space="PSUM") as accps,
            tc.tile_pool(name="onesps", bufs=1, space="PSUM") as onesps,
            tc.tile_pool(name="tailps", bufs=2, space="PSUM") as tailps,
        ):
            # ---- constants ----
            b_sb = consts.tile([P, KC, FC], f16)
            nc.scalar.dma_start(b_sb[:], B_d.rearrange("(kc p) f -> p kc f", p=P))
            ws_sb = consts.tile([P, KC], f16)
            nc.scalar.dma_start(ws_sb[:], ws_d.rearrange("(kc p) o -> p (kc o)", p=P))
            ident = consts.tile([P, P], f32)
            make_identity(nc, ident)

            # xT streamed just-in-time, one 512KB piece per adj batch, so
            # the prologue only needs piece 0 before compute starts
            xjits = {}

            def issue_xjit(b):
                xj = xpool.tile([P, KC, ab, P], f16, tag="xj")
                nc.gpsimd.dma_start(xj[:], xT_d[:, b])
                xjits[b] = xj

            # all Y chunks live in one tile; ones plane memset once
            ybig = consts.tile([P, jt_n, YTC], f16)
            nc.gpsimd.memset(ybig[:, :, FC:YTC], 1.0)
            sdst_f32 = consts.tile([P, jt_n, 1], f32)
            # B-path constants: exp factors of s_dst (per-partition scalars)
            # and a ones column for the +1-correction mini-matmuls
            fdst1 = consts.tile([P, jt_n, 1], f32)
            fdst2 = consts.tile([P, jt_n, 1], f32)
            ones_mv = consts.tile([P, 1], f16)
            nc.gpsimd.memset(ones_mv[:], 1.0)

            # ---- s_src broadcast [P, r]: ones(P) outer s_src(i_slice) ----
            # stationary wsb[k, m] = ws[k] for every m, so the matmul output
            # row m is s_src for all partitions m simultaneously. Emitted
            # from the driver loop after batch 0 so its xsT DMA doesn't
            # head-block the constants on the scalar ring.
            s_src = consts.tile([P, r], f16)
            esrc1 = consts.tile([P, r], bf16)
            esrc2 = consts.tile([P, r], bf16)

            def emit_s_src():
                xst = consts.tile([P, KC, r], f16)
                nc.scalar.dma_start(xst[:], xsT_d[:])
                wsb = consts.tile([P, KC, P], f16)
                for kc in range(KC):
                    nc.vector.tensor_copy(
                        wsb[:, kc, :], ws_sb[:, kc:kc + 1].to_broadcast([P, P])
                    )
                # [P, mov] PSUM scratch (1 bank) so onesps fits in the budget
                for hh in range(mh):
                    ssb_ps = sps.tile([P, mov], f32, tag="ssb")
                    for kc in range(KC):
                        nc.tensor.matmul(
                            ssb_ps[:],
                            wsb[:, kc, :],
                            xst[:, kc, hh * mov:(hh + 1) * mov],
                            start=(kc == 0),
                            stop=(kc == KC - 1),
                        )
                    nc.vector.tensor_copy(
                        s_src[:, hh * mov:(hh + 1) * mov], ssb_ps[:]
                    )
                # exp factors of s_src for the B-path rank-1 products
                nc.scalar.activation(esrc1[:], s_src[:], AF.Exp)
                nc.scalar.activation(esrc2[:], s_src[:], AF.Exp, scale=0.2)

            # ---- stage A: produce Y chunks for one batch (4 j-tiles) ----
            # Y^T directly: yt[j, c] = sum_k x[j, k] B[k, c] with the xT
            # chunk as (pre-transposed) stationary and B as the moving
            # operand -- lands in [j, col] layout, no PE transpose needed.
            def stage_a_batch(b):
                xj = xjits.pop(b)
                ytp = yps.tile([P, ab, FC], f32, tag="yps")
                for a in range(ab):
                    for kc in range(KC):
                        nc.tensor.matmul(
                            ytp[:, a, :],
                            xj[:, kc, a, :],
                            b_sb[:, kc, :],
                            start=(kc == 0),
                            stop=(kc == KC - 1),
                        )
                nc.vector.tensor_copy(
                    ybig[:, b * ab:(b + 1) * ab, 0:FC], ytp[:]
                )
                nc.vector.tensor_copy(
                    sdst_f32[:, b * ab:(b + 1) * ab, :], ytp[:, :, 0:1]
                )
                if b in b_batches:
                    sl = slice(b * ab, (b + 1) * ab)
                    nc.scalar.activation(
                        fdst1[:, sl, :], sdst_f32[:, sl, :], AF.Exp
                    )
                    nc.scalar.activation(
                        fdst2[:, sl, :], sdst_f32[:, sl, :], AF.Exp, scale=0.2
                    )

            # ---- stage B: one adj batch (ab j-tiles) of the attention ----
            acc = accps.tile([FM + 1, r], f32)
            acc1 = onesps.tile([FM + 1, 1], f32)
            adjts = {}

            def stage_b_batch(b):
                adjt = adjts.pop(b)
                if b not in b_batches:
                    # ---- A path: ACT Prelu -> DVE mask -> ACT quad-Exp ----
                    tbig = tpool.tile([P, ab, r], f16, tag="t")
                    for a in range(ab):
                        jt = b * ab + a
                        nc.scalar.activation(
                            tbig[:, a, :], s_src[:], AF.Prelu,
                            bias=sdst_f32[:, jt, :], scale=1.0,
                            alpha=LRELU_SLOPE,
                        )
                    upack = upool.tile([P, ab, r], f16, tag="u")
                    nc.vector.tensor_mul(upack[:], tbig[:], adjt[:])
                    ppack = ppool.tile([P, ab, r], bf16, tag="p")
                    nc.scalar.activation(ppack[:], upack[:], AF.Exp)
                else:
                    # ---- B path (no ACT): p-1 = max(e1*f1, e2*f2) - 1
                    # via 4x tensor_scalar products + 2x tt max/mask; the
                    # +1 every element owes is restored through acc1.
                    t1 = bpool.tile([P, ab, r], bf16, tag="t1")
                    t2 = bpool.tile([P, ab, r], bf16, tag="t2")
                    for a in range(ab):
                        jt = b * ab + a
                        nc.vector.tensor_scalar(
                            t1[:, a, :], esrc1[:], fdst1[:, jt, :], -1.0,
                            op0=OP.mult, op1=OP.add,
                        )
                        nc.vector.tensor_scalar(
                            t2[:, a, :], esrc2[:], fdst2[:, jt, :], -1.0,
                            op0=OP.mult, op1=OP.add,
                        )
                    mbig = bpool.tile([P, ab, r], bf16, tag="m")
                    nc.vector.tensor_tensor(
                        out=mbig[:], in0=t1[:], in1=t2[:], op=OP.max
                    )
                    ppack = ppool.tile([P, ab, r], bf16, tag="p")
                    nc.vector.tensor_mul(ppack[:], mbig[:], adjt[:])
                    # +1 correction: acc1 += Y_tile^T @ ones
                    for a in range(ab):
                        jt = b * ab + a
                        nc.tensor.matmul(
                            acc1[:],
                            ybig[:, jt, 1:YTC],
                            ones_mv[:],
                            start=(b == b_first and a == 0),
                            stop=(b == b_last and a == ab - 1),
                        )
                # all 8 accumulation matmuls of the batch back-to-back:
                # dense PE bursts keep the HAM clock-gate warm.
                for a in range(ab):
                    jt = b * ab + a
                    for hh in range(mh):
                        nc.tensor.matmul(
                            acc[:, hh * mov:(hh + 1) * mov],
                            ybig[:, jt, 1:YTC],
                            ppack[:, a, hh * mov:(hh + 1) * mov],
                            start=(jt == 0),
                            stop=(jt == jt_n - 1),
                        )

            # ---- fused pipeline: stage A batch b overlaps stage B on the
            # chunks produced by batch b-1; DMAs issued one batch ahead.
            def issue_adj(b):
                adjt = adjpool.tile([P, ab, r], f16, tag="adj")
                if b == 0:
                    # quarter DMAs: tile a of batch 0 unblocks as soon
                    # as its own slice lands during the early-DMA ramp
                    for a in range(ab):
                        nc.sync.dma_start(
                            adjt[:, a:a + 1, :], adj_d[:, b, a:a + 1, :])
                else:
                    nc.sync.dma_start(adjt[:], adj_d[:, b])
                adjts[b] = adjt

            issue_xjit(0)
            issue_adj(0)
            for b in range(n_ab + 1):
                if b + 1 < n_ab:
                    issue_xjit(b + 1)
                    issue_adj(b + 1)
                if b < n_ab:
                    stage_a_batch(b)
                if b == 0:
                    emit_s_src()
                if b >= 1:
                    stage_b_batch(b - 1)

            # ---- tail: transpose [65, r] -> [r, 65], divide, store ----
            acc_sb = consts.tile([P, r], f32)
            nc.gpsimd.memset(acc_sb[FM:P, :], 0.0)
            if b_batches:
                acc1_sb = consts.tile([FM + 1, 1], f32)
                nc.vector.tensor_copy(acc1_sb[:], acc1[:])
                nc.vector.tensor_scalar_add(
                    acc_sb[0:FM + 1, :], acc[:], acc1_sb[:]
                )
            else:
                nc.vector.tensor_copy(acc_sb[0:FM + 1, :], acc[:])
            out_sb = consts.tile([P, ich, FM], f32)
            for ic in range(ich):
                tp = tailps.tile([P, P], f32, tag="tp")
                nc.tensor.transpose(
                    tp[:], acc_sb[:, ic * P:(ic + 1) * P], ident[:]
                )
                rec = mpool.tile([P, 1], f32, tag="rec")
                nc.vector.reciprocal(rec[:], tp[:, FM:FM + 1])
                nc.vector.tensor_scalar_mul(out_sb[:, ic, :], tp[:, 0:FM], rec[:])
            nc.sync.dma_start(h_d.rearrange("(c p) f -> p c f", p=P), out_sb[:])

    return nc


def fold_weights(W, a):
    """Host-side weight folding: B = [W@a_dst | head-mean(W)], ws = W@a_src."""
    W = np.asarray(W, dtype=np.float32)
    a = np.asarray(a, dtype=np.float32).reshape(2 * F_OUT)
    ws = W @ a[:F_OUT]                                   # [F_IN]
    wd = W @ a[F_OUT:]                                   # [F_IN]
    Wm = W.reshape(F_IN, HEADS, FM).mean(axis=1)         # [F_IN, FM]
    B = np.ascontiguousarray(
        np.concatenate([wd[:, None], Wm], axis=1)
    ).astype(F16)
    return B, np.ascontiguousarray(ws[:, None]).astype(F16)


def shard_inputs(x, adj, W, a, n_cores=N_CORES):
    """Build the per-core input maps."""
    x = np.asarray(x, dtype=np.float32)
    n = x.shape[0]
    r = n // n_cores
    jt_n = n // P
    n_ab = jt_n // 4
    B, wsv = fold_weights(W, a)
    xbf = x.astype(F16)
    # xT stationary chunks: [p, b, kc, a, jj] = x[(b*4+a)*P+jj, kc*P+p]
    xT = np.ascontiguousarray(
        xbf.reshape(n_ab, 4, P, KC, P).transpose(4, 0, 3, 1, 2))
    adj8 = np.asarray(adj).astype(F16)
    in_maps = []
    for c in range(n_cores):
        i0 = c * r
        xs = xbf[i0:i0 + r]                              # [r, F_IN]
        xsT = np.ascontiguousarray(xs.reshape(r, KC, P).transpose(2, 1, 0))
        # device layout is [j (partitions), i (free)] and the attention
        # mask for output row i, summed index j is adj[i, j] -> transpose
        adjT = np.ascontiguousarray(adj8[i0:i0 + r, :].T)  # [n, r]
        ab = 4
        adjr = np.ascontiguousarray(
            adjT.reshape(jt_n // ab, ab, P, r).transpose(2, 0, 1, 3))
        m = {
            "xsT": xsT,
            "adjc": adjr,
            "B": B,
            "wsv": wsv,
            "xT": xT,
        }
        in_maps.append(m)
    return in_maps


def run(x, adj, W, a, n=N_FULL, trace=False):
    nc = build_nc(n=n)
    if not nc.is_finalized():
        nc.finalize()
    in_maps = shard_inputs(x, adj, W, a)
    core_ids = list(range(N_CORES))
    res = run_bass_kernel_spmd(nc, in_maps, core_ids, trace=trace)
    h = np.concatenate([res.results[c]["h"] for c in range(N_CORES)], axis=0)
    return h, res


def kernel(x, adj, W, a, heads=HEADS, **_ignored):
    assert int(heads) == HEADS, f"kernel hardcodes heads={HEADS}"
    assert x.shape == (N_FULL, F_IN) and adj.shape == (N_FULL, N_FULL)
    h, _ = run(x, adj, W, a, n=N_FULL, trace=False)
    return h.astype(np.float32)

